# revision 3
# baseline (speedup 1.0000x reference)
"""Trainium2 Bass kernel for nn_ActionEmbedding (B=65536, H=1024), 8-core data parallel.

Math reformulation (exact, no trig tables needed):
  reference: LayerNorm(actions[:,:3] @ pos_W + [sin(eul),cos(eul)] @ rot_W
                       + open_emb[gripper]) * ln_g + ln_b
  - sin/cos of arctan2(a,b) are a/r, b/r with r=sqrt(a^2+b^2); sin/cos of
    arcsin(c) are c, sqrt(1-c^2).  With q=(x,y,z,w) and n2=|q|^2:
      roll:  a = 2(wx+yz),          b = w^2+z^2-x^2-y^2   (1/n2 cancels)
      pitch: c = 2(wy-zx)/n2 clipped to [-1,1]
      yaw:   d = 2(wz+xy),          e = w^2+x^2-y^2-z^2
  - feature vector f = [px,py,pz, sin_r,sin_p,sin_y, cos_r,cos_p,cos_y, g, 1]
    (11) and host-precomputed W[11,1024] give h = f @ W in one matmul.
  - W rows are mean-centered over H on the host, so mean_H(h) == 0 and the
    LayerNorm mean subtraction disappears.  ln_g is folded into W.
  - row variance = f @ (W0 @ W0.T) @ f^T / H, computed on-device via a tiny
    [11,11] matmul sharing the same stationary f^T.
"""

import numpy as np
from contextlib import ExitStack

from concourse import bacc, tile
import concourse.bass as bass
import concourse.mybir as mybir
from concourse.bass_utils import run_bass_kernel_spmd

F32 = mybir.dt.float32
B, H = 65536, 1024
NCORES = 8
R = B // NCORES          # rows per core = 8192
P = 128                  # partitions
NG = R // P              # groups per core = 64
NF = 11                  # feature count

_cached = {}


def _build_graph(stage: float = 2):
    nc = bacc.Bacc(None, target_bir_lowering=False, debug=False)

    act = nc.declare_dram_parameter("actions", [R, 8], F32, isOutput=False)
    wg = nc.declare_dram_parameter("wg", [NF, H], F32, isOutput=False)
    mq = nc.declare_dram_parameter("mq", [NF, NF], F32, isOutput=False)
    idp = nc.declare_dram_parameter("ident", [P, P], F32, isOutput=False)
    out = nc.declare_dram_parameter("out", [R, H], F32, isOutput=True)

    with tile.TileContext(nc) as tc, ExitStack() as ctx:
        const = ctx.enter_context(tc.tile_pool(name="const", bufs=1))
        fpool = ctx.enter_context(tc.tile_pool(name="feat", bufs=1))
        ftp = ctx.enter_context(tc.tile_pool(name="ftp", bufs=4))
        smalls = ctx.enter_context(tc.tile_pool(name="smalls", bufs=4))
        outp = ctx.enter_context(tc.tile_pool(name="outp", bufs=4))
        psT = ctx.enter_context(tc.tile_pool(name="psT", bufs=2, space="PSUM"))
        psU = ctx.enter_context(tc.tile_pool(name="psU", bufs=2, space="PSUM"))
        psH = ctx.enter_context(tc.tile_pool(name="psH", bufs=4, space="PSUM"))

        ident = const.tile([P, P], F32)
        nc.sync.dma_start(out=ident[:], in_=idp[:])

        wg_sb = const.tile([NF, H], F32)
        nc.sync.dma_start(out=wg_sb[:], in_=wg[:])
        mq_sb = const.tile([NF, NF], F32)
        nc.sync.dma_start(out=mq_sb[:], in_=mq[:])

        # actions laid out [p, n, k]: DRAM row r = p*NG + n  (2KiB contiguous
        # per partition on load; groups are n-slices).
        a = const.tile([P, NG, 8], F32)
        nc.sync.dma_start(out=a[:, :, :],
                          in_=act[:].rearrange("(p n) k -> p n k", p=P))

        Fall = const.tile([P, NG, NF], F32)

        X, Y, Z, W = (a[:, :, 3 + i] for i in range(4))

        # scratch tiles, unique tags so each gets its own slot
        names = ("xx yy zz ww p1 p2 q1 q2 n2 b e m1 m2 ah m3 m4 dh m5 m6 ch "
                 "invn2 craw cc omc aa bb s1 r1 dd ee s2 r2").split()
        S = {nm: fpool.tile([P, NG], F32, tag=nm, name=nm) for nm in names}

        v = nc.vector
        sc = nc.scalar
        mul, add, sub = (mybir.AluOpType.mult, mybir.AluOpType.add,
                         mybir.AluOpType.subtract)

        v.tensor_mul(S["xx"][:], X, X)
        v.tensor_mul(S["yy"][:], Y, Y)
        v.tensor_mul(S["zz"][:], Z, Z)
        v.tensor_mul(S["ww"][:], W, W)
        v.tensor_add(S["p1"][:], S["ww"][:], S["zz"][:])
        v.tensor_add(S["p2"][:], S["xx"][:], S["yy"][:])
        v.tensor_add(S["q1"][:], S["ww"][:], S["xx"][:])
        v.tensor_add(S["q2"][:], S["yy"][:], S["zz"][:])
        v.tensor_add(S["n2"][:], S["p1"][:], S["p2"][:])
        v.tensor_sub(S["b"][:], S["p1"][:], S["p2"][:])
        v.tensor_sub(S["e"][:], S["q1"][:], S["q2"][:])
        # roll numerator half: ah = wx + yz
        v.tensor_mul(S["m1"][:], W, X)
        v.tensor_mul(S["m2"][:], Y, Z)
        v.tensor_add(S["ah"][:], S["m1"][:], S["m2"][:])
        # yaw numerator half: dh = wz + xy
        v.tensor_mul(S["m3"][:], W, Z)
        v.tensor_mul(S["m4"][:], X, Y)
        v.tensor_add(S["dh"][:], S["m3"][:], S["m4"][:])
        # pitch numerator half: ch = wy - zx
        v.tensor_mul(S["m5"][:], W, Y)
        v.tensor_mul(S["m6"][:], Z, X)
        v.tensor_sub(S["ch"][:], S["m5"][:], S["m6"][:])
        # sin(pitch) = clip(2*ch/n2, -1, 1)
        v.reciprocal(S["invn2"][:], S["n2"][:])
        v.scalar_tensor_tensor(S["craw"][:], S["ch"][:], 2.0, S["invn2"][:],
                               op0=mul, op1=mul)
        v.tensor_scalar(Fall[:, :, 4], S["craw"][:], 1.0, -1.0,
                        op0=mybir.AluOpType.min, op1=mybir.AluOpType.max)
        # cos(pitch) = sqrt(1 - c^2)
        v.tensor_mul(S["cc"][:], Fall[:, :, 4], Fall[:, :, 4])
        v.tensor_scalar(S["omc"][:], S["cc"][:], -1.0, 1.0, op0=mul, op1=add)
        sc.sqrt(Fall[:, :, 7], S["omc"][:])
        # roll: rinv = 1/sqrt(4*ah^2 + b^2); sin = 2*ah*rinv, cos = b*rinv
        v.tensor_mul(S["aa"][:], S["ah"][:], S["ah"][:])
        v.tensor_mul(S["bb"][:], S["b"][:], S["b"][:])
        v.scalar_tensor_tensor(S["s1"][:], S["aa"][:], 4.0, S["bb"][:],
                               op0=mul, op1=add)
        v.reciprocal(S["r1"][:], S["s1"][:])
        sc.sqrt(S["r1"][:], S["r1"][:])
        v.scalar_tensor_tensor(Fall[:, :, 3], S["ah"][:], 2.0, S["r1"][:],
                               op0=mul, op1=mul)
        v.tensor_mul(Fall[:, :, 6], S["b"][:], S["r1"][:])
        # yaw: same with dh, e
        v.tensor_mul(S["dd"][:], S["dh"][:], S["dh"][:])
        v.tensor_mul(S["ee"][:], S["e"][:], S["e"][:])
        v.scalar_tensor_tensor(S["s2"][:], S["dd"][:], 4.0, S["ee"][:],
                               op0=mul, op1=add)
        v.reciprocal(S["r2"][:], S["s2"][:])
        sc.sqrt(S["r2"][:], S["r2"][:])
        v.scalar_tensor_tensor(Fall[:, :, 5], S["dh"][:], 2.0, S["r2"][:],
                               op0=mul, op1=mul)
        v.tensor_mul(Fall[:, :, 8], S["e"][:], S["r2"][:])
        # pos, gripper, const-1 features
        v.tensor_copy(Fall[:, :, 0:3], a[:, :, 0:3])
        v.tensor_copy(Fall[:, :, 9], a[:, :, 7])
        # const-1 feature without memset (vector.memset hits a runtime bug):
        # ones = a*0 + 1 (actions are finite, so exact)
        v.tensor_scalar(Fall[:, :, 10], a[:, :, 0], 0.0, 1.0, op0=mul, op1=add)

        # [P,1] tile of the LN epsilon for the sqrt bias (a*0 + eps)
        epsb = const.tile([P, 1], F32)
        v.tensor_scalar(epsb[:], a[:, 0:1, 0], 0.0, 1e-12, op0=mul, op1=add)

        out_view = out[:].rearrange("(p n) h -> n p h", p=P)

        if stage == 1:
            # dump features for HW bisection: out[0:128, 0:704] = Fall
            dump = outp.tile([P, NG * NF], F32, tag="dump")
            v.tensor_copy(dump[:], Fall[:, :, :])
            nc.sync.dma_start(out=out[0:P, 0:NG * NF], in_=dump[:])

        for n in range(NG if stage >= 1.5 else 0):
            fT = ftp.tile([NF, P], F32, tag="fT")
            pT = psT.tile([NF, P], F32, tag="pT")
            nc.tensor.transpose(pT[:], Fall[:, n, :], ident[:])
            v.tensor_copy(fT[:], pT[:])

            pU = psU.tile([P, NF], F32, tag="pU")
            nc.tensor.matmul(pU[:], fT[:], mq_sb[:], start=True, stop=True)
            pH0 = psH.tile([P, 512], F32, tag="pH")
            pH1 = psH.tile([P, 512], F32, tag="pH")
            nc.tensor.matmul(pH0[:], fT[:], wg_sb[:, 0:512], start=True,
                             stop=True)
            nc.tensor.matmul(pH1[:], fT[:], wg_sb[:, 512:1024], start=True,
                             stop=True)

            osb = outp.tile([P, H], F32, tag="osb")
            if stage == 1.5:
                # PE-only bisect: skip stats, copy raw matmul results out
                v.tensor_copy(osb[:, 0:512], pH0[:])
                v.tensor_copy(osb[:, 512:1024], pH1[:])
                nc.sync.dma_start(out=out_view[n], in_=osb[:])
                continue
            if stage == 1.8:
                # test per-partition tensor_scalar with AP scalar, dummy rstd
                rstd = smalls.tile([P, 1], F32, tag="rstd")
                v.tensor_scalar(rstd[:], Fall[:, n, 10:11], 0.0, 0.5,
                                op0=mul, op1=add)
                v.tensor_scalar_mul(osb[:, 0:512], pH0[:], rstd[:])
                v.tensor_scalar_mul(osb[:, 512:1024], pH1[:], rstd[:])
                nc.sync.dma_start(out=out_view[n], in_=osb[:])
                continue

            trash = smalls.tile([P, NF], F32, tag="trash")
            varv = smalls.tile([P, 1], F32, tag="varv")
            v.tensor_mul(trash[:], pU[:], Fall[:, n, :])
            v.tensor_reduce(varv[:], trash[:], axis=mybir.AxisListType.X,
                            op=mybir.AluOpType.add)
            # sq = sqrt(ss/H + eps) via the activation's free affine
            sq = smalls.tile([P, 1], F32, tag="sq")
            sc.activation(sq[:], varv[:], mybir.ActivationFunctionType.Sqrt,
                          bias=epsb[:], scale=1.0 / H)
            rstd = smalls.tile([P, 1], F32, tag="rstd")
            v.reciprocal(rstd[:], sq[:])

            if stage == 1.7:
                # test ttr/sqrt/recip only; plain copies out
                v.tensor_copy(osb[:, 0:512], pH0[:])
                v.tensor_copy(osb[:, 512:1024], pH1[:])
                nc.sync.dma_start(out=out_view[n], in_=osb[:])
                continue

            v.tensor_scalar_mul(osb[:, 0:512], pH0[:], rstd[:])
            if stage >= 3:
                sc.activation(osb[:, 512:1024], pH1[:],
                              mybir.ActivationFunctionType.Copy, scale=rstd[:])
            else:
                v.tensor_scalar_mul(osb[:, 512:1024], pH1[:], rstd[:])
            nc.sync.dma_start(out=out_view[n], in_=osb[:])

    nc.finalize()
    return nc


def _host_weights(pos_W, pos_b, rot_W, rot_b, open_emb, ln_g, ln_b):
    Wf = np.zeros((NF, H), np.float64)
    Wf[0:3] = pos_W
    Wf[3:9] = rot_W
    Wf[9] = open_emb[1].astype(np.float64) - open_emb[0].astype(np.float64)
    Wf[10] = (pos_b.astype(np.float64) + rot_b.astype(np.float64)
              + open_emb[0].astype(np.float64))
    W0 = Wf - Wf.mean(axis=1, keepdims=True)
    M = W0 @ W0.T
    Wg = W0 * ln_g.astype(np.float64)[None, :]
    return Wg.astype(np.float32), M.astype(np.float32)


def kernel(_trace=False, **inputs):
    actions = np.ascontiguousarray(np.asarray(inputs["actions"], np.float32))
    Wg, M = _host_weights(
        np.asarray(inputs["pos_W"], np.float32),
        np.asarray(inputs["pos_b"], np.float32),
        np.asarray(inputs["rot_W"], np.float32),
        np.asarray(inputs["rot_b"], np.float32),
        np.asarray(inputs["open_emb"], np.float32),
        np.asarray(inputs["ln_g"], np.float32),
        np.asarray(inputs["ln_b"], np.float32),
    )

    if "nc" not in _cached:
        _cached["nc"] = _build_graph()
    nc = _cached["nc"]

    shards = actions.reshape(NCORES, R, 8)
    ident = np.eye(P, dtype=np.float32)
    in_maps = [{"actions": np.ascontiguousarray(shards[i]), "wg": Wg, "mq": M,
                "ident": ident}
               for i in range(NCORES)]
    res = run_bass_kernel_spmd(
        nc, in_maps, core_ids=list(range(NCORES)),
        trace=bool(_trace),
        trace_cores=list(range(NCORES)) if _trace else None,
    )
    _cached["last_res"] = res
    out = np.concatenate([res.results[i]["out"] for i in range(NCORES)],
                         axis=0)

    ln_b = np.asarray(inputs["ln_b"], np.float32)
    if np.any(ln_b):
        out = out + ln_b[None, :]
    return out.astype(np.float32)



# revision 11
# speedup vs baseline: 3.3100x; 3.3100x over previous
"""Trainium2 Bass kernel for nn_ActionEmbedding (B=65536, H=1024), 8-core data parallel.

Math reformulation (exact, no trig tables needed):
  reference: LayerNorm(actions[:,:3] @ pos_W + [sin(eul),cos(eul)] @ rot_W
                       + open_emb[gripper]) * ln_g + ln_b
  - sin/cos of arctan2(a,b) are a/r, b/r with r=sqrt(a^2+b^2); sin/cos of
    arcsin(c) are c, sqrt(1-c^2).  With q=(x,y,z,w) and n2=|q|^2:
      roll:  a = 2(wx+yz),          b = w^2+z^2-x^2-y^2   (1/n2 cancels)
      pitch: c = 2(wy-zx)/n2 clipped to [-1,1]
      yaw:   d = 2(wz+xy),          e = w^2+x^2-y^2-z^2
  - feature vector f = [px,py,pz, sin_r,sin_p,sin_y, cos_r,cos_p,cos_y, g, 1]
    (11) and host-precomputed W[11,1024] give h = f @ W in one matmul.
  - W rows are mean-centered over H on the host, so mean_H(h) == 0 and the
    LayerNorm mean subtraction disappears.  ln_g is folded into W.
  - row variance = f @ (W0 @ W0.T) @ f^T / H via a tiny [11,11] matmul
    sharing the same stationary f^T.

Performance structure (per core: 8192 rows = 64 groups of 128):
  - All matmul traffic is bf16 (weights, features, identity); rel err vs the
    f64 oracle is ~5e-3, well under the 2e-2 gate.
  - Features are built fp32 on Vector in [128, 64]-wide ops, written bf16
    into a 32-padded layout Fallb[128, 64, 32] so one PE transpose covers 4
    groups and lands each group's f^T at partition offset {0,32,64,96} --
    directly usable as a matmul stationary via tile_position=(32j, 0).
  - 8-group windows: variance matmuls + fused stt/accum stats + one batched
    sqrt + one batched reciprocal, then 2x512-col bf16 matmuls per group and
    whole-group PSUM evacuation alternating Vector/Scalar (both scale by
    rstd on the fly, writing bf16).
  - Output is stored bf16 (halves HBM store traffic); host upcasts to f32.
"""

import numpy as np
from contextlib import ExitStack

import ml_dtypes

from concourse import bacc, tile
import concourse.mybir as mybir
from concourse.bass_utils import run_bass_kernel_spmd

F32 = mybir.dt.float32
BF16 = mybir.dt.bfloat16
B, H = 65536, 1024
NCORES = 8
R = B // NCORES          # rows per core = 8192
P = 128                  # partitions
NG = R // P              # groups per core = 64
NF = 11                  # feature count
FPAD = 32                # padded feature stride (alignment for tile_position)
WG = 8                   # groups per window (one output DMA per window)
NW = NG // WG            # windows = 8

_cached = {}


def _build_graph():
    nc = bacc.Bacc(None, target_bir_lowering=False, debug=False)

    # wgb/mqb carry W / M replicated at partition offsets {0,32,64,96}: a
    # matmul's moving and stationary operands must start at the same SBUF
    # partition, and the stationary f^T for group j sits at partition 32*j.
    act = nc.declare_dram_parameter("actions", [R, 8], F32, isOutput=False)
    wg = nc.declare_dram_parameter("wgb", [P, H], BF16, isOutput=False)
    mq = nc.declare_dram_parameter("mqb", [P, NF], BF16, isOutput=False)
    idp = nc.declare_dram_parameter("identb", [P, P], BF16, isOutput=False)
    out = nc.declare_dram_parameter("out", [R, H], BF16, isOutput=True)

    mul, add, sub = (mybir.AluOpType.mult, mybir.AluOpType.add,
                     mybir.AluOpType.subtract)

    with tile.TileContext(nc) as tc, ExitStack() as ctx:
        const = ctx.enter_context(tc.tile_pool(name="const", bufs=1))
        fpool = ctx.enter_context(tc.tile_pool(name="feat", bufs=1))
        ftp = ctx.enter_context(tc.tile_pool(name="ftp", bufs=4))
        smalls = ctx.enter_context(tc.tile_pool(name="smalls", bufs=2))
        outp = ctx.enter_context(tc.tile_pool(name="outp", bufs=2))
        psT = ctx.enter_context(tc.tile_pool(name="psT", bufs=2, space="PSUM"))
        psU = ctx.enter_context(tc.tile_pool(name="psU", bufs=2, space="PSUM"))
        psH = ctx.enter_context(tc.tile_pool(name="psH", bufs=2, space="PSUM"))

        ident = const.tile([P, P], BF16)
        nc.sync.dma_start(out=ident[:], in_=idp[:])
        wg_sb = const.tile([P, H], BF16)
        nc.sync.dma_start(out=wg_sb[:], in_=wg[:])
        mq_sb = const.tile([P, NF], BF16)
        nc.sync.dma_start(out=mq_sb[:], in_=mq[:])

        # actions laid out [p, n, k]: DRAM row r = p*NG + n  (2KiB contiguous
        # per partition on load; groups are n-slices).
        a = const.tile([P, NG, 8], F32)
        nc.sync.dma_start(out=a[:, :, :],
                          in_=act[:].rearrange("(p n) k -> p n k", p=P))

        # bf16 feature tile, 32-padded per group: group n's features live in
        # cols [n*32, n*32+11).  Pad cols are zeroed so the PE transpose never
        # streams NaN bit patterns.
        Fallb = fpool.tile([P, NG, FPAD], BF16)
        nc.scalar.memzero(Fallb[:])

        X, Y, Z, W = (a[:, :, 3 + i] for i in range(4))

        names = ("xx yy zz ww p1 p2 q1 q2 n2 b e m1 m2 ah m3 m4 dh m5 m6 ch "
                 "invn2 craw cc omc aa bb s1 r1 dd ee s2 r2").split()
        S = {nm: fpool.tile([P, NG], F32, tag=nm, name=nm) for nm in names}

        v = nc.vector
        sc = nc.scalar

        v.tensor_mul(S["xx"][:], X, X)
        v.tensor_mul(S["yy"][:], Y, Y)
        v.tensor_mul(S["zz"][:], Z, Z)
        v.tensor_mul(S["ww"][:], W, W)
        v.tensor_add(S["p1"][:], S["ww"][:], S["zz"][:])
        v.tensor_add(S["p2"][:], S["xx"][:], S["yy"][:])
        v.tensor_add(S["q1"][:], S["ww"][:], S["xx"][:])
        v.tensor_add(S["q2"][:], S["yy"][:], S["zz"][:])
        v.tensor_add(S["n2"][:], S["p1"][:], S["p2"][:])
        v.tensor_sub(S["b"][:], S["p1"][:], S["p2"][:])
        v.tensor_sub(S["e"][:], S["q1"][:], S["q2"][:])
        # roll numerator half: ah = wx + yz
        v.tensor_mul(S["m1"][:], W, X)
        v.tensor_mul(S["m2"][:], Y, Z)
        v.tensor_add(S["ah"][:], S["m1"][:], S["m2"][:])
        # yaw numerator half: dh = wz + xy
        v.tensor_mul(S["m3"][:], W, Z)
        v.tensor_mul(S["m4"][:], X, Y)
        v.tensor_add(S["dh"][:], S["m3"][:], S["m4"][:])
        # pitch numerator half: ch = wy - zx
        v.tensor_mul(S["m5"][:], W, Y)
        v.tensor_mul(S["m6"][:], Z, X)
        v.tensor_sub(S["ch"][:], S["m5"][:], S["m6"][:])
        # sin(pitch) = clip(2*ch/n2, -1, 1)
        v.reciprocal(S["invn2"][:], S["n2"][:])
        v.scalar_tensor_tensor(S["craw"][:], S["ch"][:], 2.0, S["invn2"][:],
                               op0=mul, op1=mul)
        v.tensor_scalar(Fallb[:, :, 4], S["craw"][:], 1.0, -1.0,
                        op0=mybir.AluOpType.min, op1=mybir.AluOpType.max)
        # cos(pitch) = sqrt(1 - c^2)
        v.tensor_mul(S["cc"][:], Fallb[:, :, 4], Fallb[:, :, 4])
        v.tensor_scalar(S["omc"][:], S["cc"][:], -1.0, 1.0, op0=mul, op1=add)
        sc.sqrt(Fallb[:, :, 7], S["omc"][:])
        # roll: rinv = 1/sqrt(4*ah^2 + b^2); sin = 2*ah*rinv, cos = b*rinv
        v.tensor_mul(S["aa"][:], S["ah"][:], S["ah"][:])
        v.tensor_mul(S["bb"][:], S["b"][:], S["b"][:])
        v.scalar_tensor_tensor(S["s1"][:], S["aa"][:], 4.0, S["bb"][:],
                               op0=mul, op1=add)
        v.reciprocal(S["r1"][:], S["s1"][:])
        sc.sqrt(S["r1"][:], S["r1"][:])
        v.scalar_tensor_tensor(Fallb[:, :, 3], S["ah"][:], 2.0, S["r1"][:],
                               op0=mul, op1=mul)
        v.tensor_mul(Fallb[:, :, 6], S["b"][:], S["r1"][:])
        # yaw: same with dh, e
        v.tensor_mul(S["dd"][:], S["dh"][:], S["dh"][:])
        v.tensor_mul(S["ee"][:], S["e"][:], S["e"][:])
        v.scalar_tensor_tensor(S["s2"][:], S["dd"][:], 4.0, S["ee"][:],
                               op0=mul, op1=add)
        v.reciprocal(S["r2"][:], S["s2"][:])
        sc.sqrt(S["r2"][:], S["r2"][:])
        v.scalar_tensor_tensor(Fallb[:, :, 5], S["dh"][:], 2.0, S["r2"][:],
                               op0=mul, op1=mul)
        v.tensor_mul(Fallb[:, :, 8], S["e"][:], S["r2"][:])
        # pos, gripper, const-1 features
        v.tensor_copy(Fallb[:, :, 0:3], a[:, :, 0:3])
        v.tensor_copy(Fallb[:, :, 9], a[:, :, 7])
        # const-1 feature without memset (vector.memset hits a runtime bug):
        # ones = a*0 + 1 (actions are finite, so exact)
        v.tensor_scalar(Fallb[:, :, 10], a[:, :, 0], 0.0, 1.0, op0=mul,
                        op1=add)

        # [P,1] tile of the LN epsilon for the sqrt bias (a*0 + eps)
        epsb = const.tile([P, 1], F32)
        v.tensor_scalar(epsb[:], a[:, 0:1, 0], 0.0, 1e-12, op0=mul, op1=add)

        # output rows: DRAM row r = p*NG + w*WG + j
        out_view = out[:].rearrange("(p w j) h -> w p (j h)", p=P, j=WG)

        fT_of = {}  # 4-group chunk index -> SBUF f^T tile [128, 128]

        for w in range(NW):
            # ---- phase A: transposes, variance matmuls, batched stats ----
            for t in range(2):
                i = w * 2 + t
                pT = psT.tile([P, P], BF16, tag="pT")
                nc.tensor.transpose(pT[:], Fallb[:, 4 * i:4 * i + 4, :],
                                    ident[:])
                fT = ftp.tile([P, P], BF16, tag="fT")
                v.tensor_copy(fT[:], pT[:])
                fT_of[i] = fT

            pU = psU.tile([P, WG, 16], F32, tag="pU")
            varv = smalls.tile([P, WG], F32, tag="varv")
            trash = smalls.tile([P, NF], F32, tag="trash")
            for j in range(WG):
                n = w * WG + j
                fT = fT_of[n // 4]
                blk = FPAD * (n % 4)
                nc.tensor.matmul(pU[:, j, 0:NF], fT[blk:blk + NF, :],
                                 mq_sb[blk:blk + NF, :], start=True,
                                 stop=True, tile_position=(blk, 0))
                v.scalar_tensor_tensor(trash[:], pU[:, j, 0:NF], 1.0,
                                       Fallb[:, n, 0:NF], op0=mul, op1=mul,
                                       accum_out=varv[:, j:j + 1])
            sq = smalls.tile([P, WG], F32, tag="sq")
            sc.activation(sq[:], varv[:], mybir.ActivationFunctionType.Sqrt,
                          bias=epsb[:], scale=1.0 / H)
            rstd = smalls.tile([P, WG], F32, tag="rstd")
            v.reciprocal(rstd[:], sq[:])

            # ---- phase B: main matmuls + whole-group evacuation ----
            osb = outp.tile([P, WG, H], BF16, tag="osb")
            for j in range(WG):
                n = w * WG + j
                fT = fT_of[n // 4]
                blk = FPAD * (n % 4)
                pH = psH.tile([P, H], F32, tag="pH")
                nc.tensor.matmul(pH[:, 0:512], fT[blk:blk + NF, :],
                                 wg_sb[blk:blk + NF, 0:512], start=True,
                                 stop=True, tile_position=(blk, 0))
                nc.tensor.matmul(pH[:, 512:1024], fT[blk:blk + NF, :],
                                 wg_sb[blk:blk + NF, 512:1024], start=True,
                                 stop=True, tile_position=(blk, 0))
                if j % 2 == 0:
                    v.tensor_scalar_mul(osb[:, j, :], pH[:],
                                        rstd[:, j:j + 1])
                else:
                    sc.activation(osb[:, j, :], pH[:],
                                  mybir.ActivationFunctionType.Copy,
                                  scale=rstd[:, j:j + 1])
            nc.sync.dma_start(out=out_view[w], in_=osb[:])

    nc.finalize()
    return nc


def _host_weights(pos_W, pos_b, rot_W, rot_b, open_emb, ln_g, ln_b):
    Wf = np.zeros((NF, H), np.float64)
    Wf[0:3] = pos_W
    Wf[3:9] = rot_W
    Wf[9] = open_emb[1].astype(np.float64) - open_emb[0].astype(np.float64)
    Wf[10] = (pos_b.astype(np.float64) + rot_b.astype(np.float64)
              + open_emb[0].astype(np.float64))
    W0 = Wf - Wf.mean(axis=1, keepdims=True)
    M = W0 @ W0.T
    Wg = W0 * ln_g.astype(np.float64)[None, :]
    # replicate at partition offsets {0,32,64,96} (see _build_graph)
    Wg4 = np.zeros((P, H), np.float64)
    M4 = np.zeros((P, NF), np.float64)
    for blk in range(0, P, FPAD):
        Wg4[blk:blk + NF] = Wg
        M4[blk:blk + NF] = M
    return (Wg4.astype(ml_dtypes.bfloat16), M4.astype(ml_dtypes.bfloat16))


def kernel(_trace=False, **inputs):
    actions = np.ascontiguousarray(np.asarray(inputs["actions"], np.float32))
    Wgb, Mb = _host_weights(
        np.asarray(inputs["pos_W"], np.float32),
        np.asarray(inputs["pos_b"], np.float32),
        np.asarray(inputs["rot_W"], np.float32),
        np.asarray(inputs["rot_b"], np.float32),
        np.asarray(inputs["open_emb"], np.float32),
        np.asarray(inputs["ln_g"], np.float32),
        np.asarray(inputs["ln_b"], np.float32),
    )

    if "nc" not in _cached:
        _cached["nc"] = _build_graph()
    nc = _cached["nc"]

    shards = actions.reshape(NCORES, R, 8)
    identb = np.eye(P, dtype=ml_dtypes.bfloat16)
    in_maps = [{"actions": np.ascontiguousarray(shards[i]), "wgb": Wgb,
                "mqb": Mb, "identb": identb}
               for i in range(NCORES)]
    res = run_bass_kernel_spmd(
        nc, in_maps, core_ids=list(range(NCORES)),
        trace=bool(_trace),
        trace_cores=list(range(NCORES)) if _trace else None,
    )
    _cached["last_res"] = res
    out = np.concatenate([res.results[i]["out"] for i in range(NCORES)],
                         axis=0).astype(np.float32)

    ln_b = np.asarray(inputs["ln_b"], np.float32)
    if np.any(ln_b):
        out = out + ln_b[None, :]
    return out
